# revision 26
# baseline (speedup 1.0000x reference)
"""BalancedMoE (B=8192, D=2048, E=8, top-2) on 8 Trainium2 NeuronCores.

Strategy: expert-parallel with host-side sparse dispatch + 2-weight-set
load balancing.
  - Host computes gate logits / top-2 routing / softmax gates, gathers
    each expert's tokens and transposes them into [D, C] so the device
    needs no on-chip transposes.
  - Plain expert-parallel pads every core to the BIGGEST expert's token
    count (C_max=2234 here vs mean 2048). Instead each core holds TWO
    expert weight sets in SBUF and processes two token slots (a, b); a
    small host-side solver splits oversized experts across cores so the
    per-core column count drops to ~max(C_max/2 rounded pairings)
    (2113 here) — a ~5% compute-floor cut.
  - bf16 operands/outputs: matmul rate is identical to fp32r (1 col/
    cycle) but halves every DMA stream; rel-err ~2.7e-3 vs the 2e-2 gate.
  - Host scatters the per-(core,slot) outputs back and combines with
    the gate weights.

Per-core Bass kernel: outT[o, t] = sum_d W_s[o, d] * toks[t, d] + b_s[o]
with s = the slot (weight set) the column t belongs to.
  lhsT = W_s^T tiles (stationary), rhs = toksT tiles (moving).
"""

import os

import numpy as np

P = 128
B = 8192
D_LAT = 1024
D_EMB = 1024
D = D_LAT + D_EMB  # 2048
E = 8
TOPK = 2
N_CORES = 8


# ----------------------------------------------------------------- device ---

_cache = {}


def _ntff_shim():
    """Register the axon NTFF profile hook that the boot skips when
    antenv.axon_hooks is missing (so BASS_TRACE=1 yields exec_time_ns)."""
    import sys
    import types

    if "antenv.axon_hooks" in sys.modules:
        return
    holder = [None]
    mod = types.ModuleType("antenv.axon_hooks")
    mod.set_axon_ntff_profile_hook = lambda h: holder.__setitem__(0, h)
    mod.get_axon_ntff_profile_hook = lambda: holder[0]
    sys.modules["antenv.axon_hooks"] = mod
    try:
        import antenv

        antenv.axon_hooks = mod
        from trn_agent_boot.trn_boot import _ntff_profile_via_ctypes

        mod.set_axon_ntff_profile_hook(
            _ntff_profile_via_ctypes("/opt/axon/libaxon_pjrt.so")
        )
    except Exception:
        pass


def _split_slot(C, lead_small=False, tail_small=False):
    """Tile widths for one slot. bf16 runs at 1 col/cycle at any width.
    lead_small: 384-col first tile so the first matmuls need only a small
    token download (startup is DMA-latency-bound) while the weight-chunk
    deadlines (one per 2.6us) stay behind the Act-queue stream.
    tail_small: keep the final tile <= 256 so the end-of-kernel drain
    (last vector op + last output DMA) is short."""
    tiles = []
    if lead_small and C >= 896:
        tiles.append(384)
        C -= 384
    while C > 512:
        tiles.append(512)
        C -= 512
    if tail_small and C > 256:
        tiles.extend([C - 186, 186])
    elif C:
        tiles.append(C)
    return tiles


# Weight m-chunk batches (all on the Act HWDGE queue; SP carries only
# tokens/bias/outputs). DMA COUNT IS PRECIOUS: the hardware recycles ~10
# DGE semaphores across both queues, and a trigger whose semaphore is
# still held by an in-flight DMA stalls its whole sequencer stream. So:
# few batches, sized so chunk m lands before its ~2.6us-spaced deadline.
_W_SET0 = [(0, 1), (1, 3), (3, 6), (6, 10), (10, 16)]
_W_SET1 = [(0, 6), (6, 11), (11, 16)]


def _build(Ca, Cb, dt_name):
    import concourse.mybir as mybir
    from concourse import bacc
    from concourse.bass import ds
    from concourse.tile import TileContext

    dt_in = getattr(mybir.dt, dt_name)
    dt_out = mybir.dt.bfloat16 if dt_name == "bfloat16" else mybir.dt.float32
    KT = D // P
    MT = D // P
    C = Ca + Cb
    # (size, weight-set) per moving tile; slot-a tiles first so set 1 isn't
    # needed until ~half way through the kernel (its DMA has ~100us of slack)
    tiles = [(sz, 0) for sz in _split_slot(Ca, lead_small=True)]
    tiles += [(sz, 1) for sz in _split_slot(Cb, tail_small=True)]
    nsets = 2 if Cb else 1

    nc = bacc.Bacc(
        "TRN2", target_bir_lowering=False, debug=False, num_devices=N_CORES
    )
    # ALL big tensors are stored in DRAM in the exact SBUF block layout
    # (partition-major, tile-blocked). DMA bandwidth under queue contention
    # is proportional to the contiguous-run length: with these layouts every
    # stream moves 16-24KB/partition runs (vs 1KB for a naive [D, C] layout,
    # which starved the token queue to ~80GB/s while weights ran 350GB/s).
    #   wp[s, ki, m, ko, o]        = W_set_s[m*128 + o, ko*128 + ki]
    #   tokp[tile n: ki, ko, t]    = token_t_of_tile_n[ko*128 + ki]
    #   outp[tile n: mi, mo, t]    = out_t_of_tile_n[mo*128 + mi]
    wp = nc.dram_tensor("wp", [nsets, P, MT, KT, P], dt_in, kind="ExternalInput")
    tokp = nc.dram_tensor("tokp", [D * C], dt_in, kind="ExternalInput")
    # bias partition-major: biasp[mi, s, mo] = b_set_s[mo*128 + mi]. The
    # naive [nsets, D] layout lowers to 4096 FOUR-BYTE DMA packets (~37us
    # of queue time) that poison the token queue right at startup.
    bias = nc.dram_tensor(
        "bias", [P, nsets, MT], mybir.dt.float32, kind="ExternalInput"
    )
    outp = nc.dram_tensor("outp", [D * C], dt_out, kind="ExternalOutput")

    b_r = bias.ap()
    w_r = wp.ap()

    # 2-D views [partition, flat-run]: keeping the free dim FLAT (not
    # [KT, t]) lets the DMA lowering emit one 12-32KB contiguous run per
    # partition instead of KT separate sz*2B runs — packet size decides
    # each queue's share of HBM bandwidth under contention.
    def tok_block(off, sz):
        return tokp.ap()[D * off : D * (off + sz)].rearrange(
            "(ki r) -> ki r", ki=P
        )

    def out_block(off, sz):
        return outp.ap()[D * off : D * (off + sz)].rearrange(
            "(mi r) -> mi r", mi=P
        )

    with TileContext(nc) as tc:
        with (
            tc.tile_pool(name="w", bufs=1) as w_pool,
            tc.tile_pool(name="tok", bufs=2) as tok_pool,
            tc.tile_pool(name="out", bufs=2) as out_pool,
            tc.tile_pool(name="bias", bufs=1) as b_pool,
            tc.tile_pool(name="ps", bufs=8, space="PSUM") as ps_pool,
        ):
            bias_tile = b_pool.tile([P, nsets, MT], mybir.dt.float32)
            nc.sync.dma_start(bias_tile[:], b_r)
            tok_tiles = {}

            def load_toks(n, n_off, n_sz, chunks=None, eng=None):
                # flat [P, KT*sz] tile: k-slice c is columns [c*sz, (c+1)*sz)
                t_full = tok_pool.tile([P, KT * 512], dt_in, tag="tok")
                t_tile = t_full[:, : KT * n_sz]
                blk = tok_block(n_off, n_sz)
                eng = eng or nc.sync
                if chunks:
                    # k-slice chunks so the first matmuls only wait for the
                    # slices they read
                    k = 0
                    for w in chunks:
                        eng.dma_start(
                            t_tile[:, k * n_sz : (k + w) * n_sz],
                            blk[:, k * n_sz : (k + w) * n_sz],
                        )
                        k += w
                else:
                    eng.dma_start(t_tile, blk)
                tok_tiles[n] = (t_tile, n_sz)

            w_tiles = [[None] * MT for _ in range(nsets)]

            def load_w_batch(s, lo, hi):
                w_t = w_pool.tile([P, hi - lo, KT, P], dt_in, tag=f"w{s}_{lo}")
                nc.scalar.dma_start(w_t[:], w_r[s][:, lo:hi])
                for m in range(lo, hi):
                    w_tiles[s][m] = w_t[:, m - lo]

            # Startup streams: tok0's first k-chunk rides the HEAD of the
            # Act queue (lands first -> matmul 0 starts ~2us earlier), the
            # weight batches follow it on Act; the rest of tok0 streams in
            # parallel on SP (with bias), so both queues contribute to the
            # critical first ~15us.
            sz0 = tiles[0][0]
            load_toks(0, 0, sz0, chunks=(4,), eng=nc.scalar)
            load_w_batch(0, *_W_SET0[0])
            blk0 = tok_block(0, sz0)
            t0 = tok_tiles[0][0]
            nc.sync.dma_start(t0[:, 4 * sz0 : 10 * sz0], blk0[:, 4 * sz0 : 10 * sz0])
            nc.sync.dma_start(t0[:, 10 * sz0 :], blk0[:, 10 * sz0 :])
            for lo, hi in _W_SET0[1:]:
                load_w_batch(0, lo, hi)
            if nsets > 1:
                for lo, hi in _W_SET1:
                    load_w_batch(1, lo, hi)

            n_offs = []
            off = 0
            for sz, _s in tiles:
                n_offs.append(off)
                off += sz

            for n, (n_sz, s) in enumerate(tiles):
                if n >= 2 and n + 1 < len(tiles):
                    load_toks(n + 1, n_offs[n + 1], tiles[n + 1][0])
                t_tile, t_sz = tok_tiles.pop(n)
                assert t_sz == n_sz
                n_off = n_offs[n]
                o_full = out_pool.tile([P, MT * 512], dt_out, tag="out")
                o_tile = o_full[:, : MT * n_sz]
                for m in range(MT):
                    # issue the n=1 token prefetch from the middle of n=0's
                    # SP stream: early enough to land before n=1 starts,
                    # late enough not to steal HBM from the weight stream
                    if n == 0 and m == 8 and len(tiles) > 1:
                        load_toks(1, n_offs[1], tiles[1][0])
                    if n == 1 and m == 8 and len(tiles) > 2:
                        load_toks(2, n_offs[2], tiles[2][0])
                    ps_full = ps_pool.tile([P, 512], mybir.dt.float32, tag="ps")
                    ps = ps_full[:, :n_sz]
                    for k in range(KT):
                        nc.tensor.matmul(
                            ps,
                            w_tiles[s][m][:, k, :],
                            t_tile[:, k * n_sz : (k + 1) * n_sz],
                            start=(k == 0),
                            stop=(k == KT - 1),
                        )
                    nc.vector.tensor_scalar_add(
                        o_tile[:, m * n_sz : (m + 1) * n_sz],
                        ps,
                        bias_tile[:, s, m : m + 1],
                    )
                # ONE output DMA per tile (not per m): DMA count is the
                # scarce resource. The last tile's outputs go in 4 chunks
                # so the final drain pipelines behind the last matmuls.
                o_blk = out_block(n_off, n_sz)
                if n == len(tiles) - 1:
                    for m4 in range(0, MT, 4):
                        nc.sync.dma_start(
                            o_blk[:, m4 * n_sz : (m4 + 4) * n_sz],
                            o_tile[:, m4 * n_sz : (m4 + 4) * n_sz],
                        )
                else:
                    nc.sync.dma_start(o_blk, o_tile)
    nc.compile()
    return nc


def _get_program(Ca, Cb, dt_name):
    key = (Ca, Cb, dt_name)
    if key not in _cache:
        _cache[key] = _build(Ca, Cb, dt_name)
    return _cache[key]


# ------------------------------------------------------------- host: pack ---


def _solve_pack(counts):
    """2-weight-set slot packing: each core gets slot_a (Ca cols, weight
    set 0) and slot_b (Cb cols, weight set 1); every slot holds a chunk of
    ONE expert's tokens. x experts split across two a-slots, 8-2x experts
    use one core's (a,b), x experts split across two b-slots. Minimizes
    N = Ca + Cb (the per-core padded column count)."""
    n = len(counts)
    order = np.argsort(-np.asarray(counts), kind="stable")
    c = [int(counts[i]) for i in order]
    best = None
    for x in range(0, n // 2 + 1):
        if x == 0:
            a, b, N = c[0], 0, c[0]
        else:
            a = (c[0] + 1) // 2
            b = (c[n - x] + 1) // 2
            mids = c[x : n - x]
            N = max(a + b, mids[0] if mids else 0)
            a = max(a, N - b)
            b = N - a
        if best is None or N < best[0]:
            best = (N, x, a, b)
    N, x, a, b = best
    plan = [[] for _ in range(n)]  # per core: list of (slot, expert, lo, hi)
    for j in range(x):  # biggest experts -> a-slots of cores 2j, 2j+1
        e = int(order[j])
        ce = c[j]
        cut = min(a, ce)
        plan[2 * j].append(("a", e, 0, cut))
        if ce > cut:
            plan[2 * j + 1].append(("a", e, cut, ce))
    for k, j in enumerate(range(x, n - x)):  # mids -> (a,b) of one core
        e = int(order[j])
        ce = c[j]
        core = 2 * x + k
        cut = min(a, ce)
        plan[core].append(("a", e, 0, cut))
        if ce > cut:
            plan[core].append(("b", e, cut, ce))
    for j in range(x):  # smallest experts -> b-slots of cores 2j, 2j+1
        e = int(order[n - x + j])
        ce = c[n - x + j]
        cut = min(b, ce)
        plan[2 * j].append(("b", e, 0, cut))
        if ce > cut:
            plan[2 * j + 1].append(("b", e, cut, ce))
    # sanity: coverage and capacity
    cov = [0] * n
    for core, items in enumerate(plan):
        used = {"a": 0, "b": 0}
        owners = {"a": set(), "b": set()}
        for slot, e, lo, hi in items:
            used[slot] += hi - lo
            owners[slot].add(e)
            cov[e] += hi - lo
        if used["a"] > a or used["b"] > b:
            return None
        if len(owners["a"]) > 1 or len(owners["b"]) > 1:
            return None
    if cov != [int(v) for v in counts]:
        return None
    return N, x, a, b, plan


# ------------------------------------------------------------------- host ---


def kernel(x, y, W_experts, b_experts, W_gate, b_gate):
    x = np.asarray(x, dtype=np.float32)
    y = np.asarray(y, dtype=np.float32)
    W_experts = np.asarray(W_experts, dtype=np.float32)
    b_experts = np.asarray(b_experts, dtype=np.float32)
    W_gate = np.asarray(W_gate, dtype=np.float32)
    b_gate = np.asarray(b_gate, dtype=np.float32)

    inp = np.concatenate([x, y], axis=1)  # [B, D]

    # ---- routing (host) ----
    logits = inp.astype(np.float64) @ W_gate.T.astype(np.float64) + b_gate
    order = np.argsort(-logits, axis=1, kind="stable")
    top2 = order[:, :TOPK]  # [B, 2]
    v = np.take_along_axis(logits, top2, axis=1)
    v = v - v.max(axis=1, keepdims=True)
    ev = np.exp(v)
    g = (ev / ev.sum(axis=1, keepdims=True)).astype(np.float32)  # [B, 2]

    counts = np.bincount(top2.ravel(), minlength=E)

    idx_list = []
    wgt_list = []
    for e in range(E):
        m0 = top2[:, 0] == e
        m1 = top2[:, 1] == e
        idx_e = np.concatenate([np.nonzero(m0)[0], np.nonzero(m1)[0]])
        w_e = np.concatenate([g[m0, 0], g[m1, 1]])
        idx_list.append(idx_e)
        wgt_list.append(w_e)

    dt_name = os.environ.get("MOE_DT", "bfloat16")
    if dt_name == "bfloat16":
        import ml_dtypes

        np_in_dt = np.dtype(ml_dtypes.bfloat16)
    else:
        np_in_dt = np.dtype(np.float32)

    pack = None
    if os.environ.get("MOE_PACK", "1") == "1" and E == N_CORES:
        pack = _solve_pack(counts)
    if pack is not None and pack[3] > 0:
        N, _x, Ca, Cb, plan = pack
        Ca = max(Ca, 512)  # tiling floor
    else:
        Ca, Cb = max(512, int(counts.max())), 0
        plan = [[("a", e, 0, int(counts[e]))] for e in range(E)]

    inpT = np.ascontiguousarray(inp.T.astype(np_in_dt))  # [D, B]
    MT = KT = D // P
    wpacked = [None] * E

    def _wpack(e):
        if wpacked[e] is None:
            # wp[ki, m, ko, o] = W_e[m*128 + o, ko*128 + ki] (partition-major)
            wpacked[e] = np.ascontiguousarray(
                W_experts[e]
                .reshape(MT, P, KT, P)
                .transpose(3, 0, 2, 1)
                .astype(np_in_dt)
            )
        return wpacked[e]

    # device-side tile structure (must mirror _split_slot in _build)
    tile_sizes = [sz for sz in _split_slot(Ca, lead_small=True)]
    tile_offs = list(np.cumsum([0] + tile_sizes))[:-1]
    if Cb:
        b_sizes = _split_slot(Cb, tail_small=True)
        tile_offs += [Ca + o for o in np.cumsum([0] + b_sizes)[:-1]]
        tile_sizes += b_sizes

    def _tok_pack(toksT):
        # [D, C] -> tile-blocked [P, KT, sz] per tile, flattened
        blocks = []
        for off, sz in zip(tile_offs, tile_sizes):
            blk = toksT[:, off : off + sz].reshape(KT, P, sz).transpose(1, 0, 2)
            blocks.append(np.ascontiguousarray(blk).reshape(-1))
        return np.concatenate(blocks)

    nsets = 2 if Cb else 1
    slot_off = {"a": 0, "b": Ca}
    slot_idx = {"a": 0, "b": 1}
    in_maps = []
    for core in range(E):
        toksT = np.zeros((D, Ca + Cb), dtype=np_in_dt)
        wp = np.zeros((nsets, P, MT, KT, P), dtype=np_in_dt)
        bias = np.zeros((nsets, D), dtype=np.float32)
        for slot, e, lo, hi in plan[core]:
            off = slot_off[slot]
            toksT[:, off : off + (hi - lo)] = inpT[:, idx_list[e][lo:hi]]
            wp[slot_idx[slot]] = _wpack(e)
            bias[slot_idx[slot]] = b_experts[e]
        # biasp[mi, s, mo] = bias[s, mo*128 + mi] (partition-major)
        biasp = np.ascontiguousarray(
            bias.reshape(nsets, MT, P).transpose(2, 0, 1)
        )
        in_maps.append({"wp": wp, "tokp": _tok_pack(toksT), "bias": biasp})

    # ---- device ----
    if os.environ.get("BASS_TRACE"):
        _ntff_shim()
    from concourse.bass_utils import run_bass_kernel_spmd

    nc = _get_program(Ca, Cb, dt_name)
    res = None
    for attempt in range(3):
        try:
            res = run_bass_kernel_spmd(nc, in_maps, core_ids=list(range(N_CORES)))
            break
        except Exception:
            # the axon-tunneled device occasionally reports a transient
            # NRT_EXEC_UNIT_UNRECOVERABLE; it recovers after a short wait
            if attempt == 2:
                raise
            import time

            time.sleep(20 * (attempt + 1))
            try:
                import jax

                jax.clear_caches()
            except Exception:
                pass
    globals()["_last_res"] = res
    if res.exec_time_ns is not None:
        print(f"HW exec time: {res.exec_time_ns} ns")

    # ---- combine (host) ----
    fused = np.zeros((B, D), dtype=np.float32)
    for core in range(E):
        outp = res.results[core]["outp"]
        # un-block: tile n is [P, MT, sz] at offset D*tile_offs[n]
        outT = np.empty((D, Ca + Cb), dtype=np.float32)
        for off, sz in zip(tile_offs, tile_sizes):
            blk = outp[D * off : D * (off + sz)].reshape(P, MT, sz)
            outT[:, off : off + sz] = (
                blk.transpose(1, 0, 2).reshape(D, sz).astype(np.float32)
            )
        for slot, e, lo, hi in plan[core]:
            off = slot_off[slot]
            rows = outT[:, off : off + (hi - lo)].T
            fused[idx_list[e][lo:hi]] += rows * wgt_list[e][lo:hi, None]
    return fused


# revision 29
# speedup vs baseline: 1.1763x; 1.1763x over previous
"""BalancedMoE (B=8192, D=2048, E=8, top-2) on 8 Trainium2 NeuronCores.

Strategy: expert-parallel with host-side sparse dispatch + 2-weight-set
load balancing.
  - Host computes gate logits / top-2 routing / softmax gates, gathers
    each expert's tokens and transposes them into [D, C] so the device
    needs no on-chip transposes.
  - Plain expert-parallel pads every core to the BIGGEST expert's token
    count (C_max=2234 here vs mean 2048). Instead each core holds TWO
    expert weight sets in SBUF and processes two token slots (a, b); a
    small host-side solver splits oversized experts across cores so the
    per-core column count drops to ~max(C_max/2 rounded pairings)
    (2113 here) — a ~5% compute-floor cut.
  - bf16 operands/outputs: matmul rate is identical to fp32r (1 col/
    cycle) but halves every DMA stream; rel-err ~2.7e-3 vs the 2e-2 gate.
  - Host scatters the per-(core,slot) outputs back and combines with
    the gate weights.

Per-core Bass kernel: outT[o, t] = sum_d W_s[o, d] * toks[t, d] + b_s[o]
with s = the slot (weight set) the column t belongs to.
  lhsT = W_s^T tiles (stationary), rhs = toksT tiles (moving).
"""

import os

import numpy as np

P = 128
B = 8192
D_LAT = 1024
D_EMB = 1024
D = D_LAT + D_EMB  # 2048
E = 8
TOPK = 2
N_CORES = 8


# ----------------------------------------------------------------- device ---

_cache = {}


def _ntff_shim():
    """Register the axon NTFF profile hook that the boot skips when
    antenv.axon_hooks is missing (so BASS_TRACE=1 yields exec_time_ns)."""
    import sys
    import types

    if "antenv.axon_hooks" in sys.modules:
        return
    holder = [None]
    mod = types.ModuleType("antenv.axon_hooks")
    mod.set_axon_ntff_profile_hook = lambda h: holder.__setitem__(0, h)
    mod.get_axon_ntff_profile_hook = lambda: holder[0]
    sys.modules["antenv.axon_hooks"] = mod
    try:
        import antenv

        antenv.axon_hooks = mod
        from trn_agent_boot.trn_boot import _ntff_profile_via_ctypes

        mod.set_axon_ntff_profile_hook(
            _ntff_profile_via_ctypes("/opt/axon/libaxon_pjrt.so")
        )
    except Exception:
        pass


def _split_slot(C, lead_small=False, tail_small=False):
    """Tile widths for one slot. bf16 runs at 1 col/cycle at any width.
    lead_small: 384-col first tile so the first matmuls need only a small
    token download (startup is DMA-latency-bound) while the weight-chunk
    deadlines (one per 2.6us) stay behind the Act-queue stream.
    tail_small: keep the final tile <= 256 so the end-of-kernel drain
    (last vector op + last output DMA) is short."""
    tiles = []
    if lead_small and C >= 896:
        tiles.append(384)
        C -= 384
    while C > 512:
        tiles.append(512)
        C -= 512
    if tail_small and C > 256:
        tiles.extend([C - 186, 186])
    elif C:
        tiles.append(C)
    return tiles


# Weight m-chunk batches (all on the Act HWDGE queue; SP carries only
# tokens/bias/outputs). DMA COUNT IS PRECIOUS: the hardware recycles ~10
# DGE semaphores across both queues, and a trigger whose semaphore is
# still held by an in-flight DMA stalls its whole sequencer stream. So:
# few batches, sized so chunk m lands before its ~2.6us-spaced deadline.
_W_SET0 = [(0, 1), (1, 3), (3, 6), (6, 10), (10, 16)]
_W_SET1 = [(0, 6), (6, 11), (11, 16)]


def _build(Ca, Cb, dt_name):
    import concourse.mybir as mybir
    from concourse import bacc
    from concourse.bass import ds
    from concourse.tile import TileContext

    dt_in = getattr(mybir.dt, dt_name)
    dt_out = mybir.dt.bfloat16 if dt_name == "bfloat16" else mybir.dt.float32
    KT = D // P
    MT = D // P
    C = Ca + Cb
    # (size, weight-set) per moving tile; slot-a tiles first so set 1 isn't
    # needed until ~half way through the kernel (its DMA has ~100us of slack)
    tiles = [(sz, 0) for sz in _split_slot(Ca, lead_small=True)]
    tiles += [(sz, 1) for sz in _split_slot(Cb, tail_small=True)]
    nsets = 2 if Cb else 1

    nc = bacc.Bacc(
        "TRN2", target_bir_lowering=False, debug=False, num_devices=N_CORES
    )
    # ALL big tensors are stored in DRAM in the exact SBUF block layout
    # (partition-major, tile-blocked). DMA bandwidth under queue contention
    # is proportional to the contiguous-run length: with these layouts every
    # stream moves 16-24KB/partition runs (vs 1KB for a naive [D, C] layout,
    # which starved the token queue to ~80GB/s while weights ran 350GB/s).
    #   wp[s, ki, m, ko, o]        = W_set_s[m*128 + o, ko*128 + ki]
    #   tokp[tile n: ki, ko, t]    = token_t_of_tile_n[ko*128 + ki]
    #   outp[tile n: mi, mo, t]    = out_t_of_tile_n[mo*128 + mi]
    wp = nc.dram_tensor("wp", [nsets, P, MT, KT, P], dt_in, kind="ExternalInput")
    tokp = nc.dram_tensor("tokp", [D * C], dt_in, kind="ExternalInput")
    # bias partition-major: biasp[mi, s, mo] = b_set_s[mo*128 + mi]. The
    # naive [nsets, D] layout lowers to 4096 FOUR-BYTE DMA packets (~37us
    # of queue time) that poison the token queue right at startup.
    bias = nc.dram_tensor(
        "bias", [P, nsets, MT], mybir.dt.float32, kind="ExternalInput"
    )
    outp = nc.dram_tensor("outp", [D * C], dt_out, kind="ExternalOutput")

    b_r = bias.ap()
    w_r = wp.ap()

    # 2-D views [partition, flat-run]: keeping the free dim FLAT (not
    # [KT, t]) lets the DMA lowering emit one 12-32KB contiguous run per
    # partition instead of KT separate sz*2B runs — packet size decides
    # each queue's share of HBM bandwidth under contention.
    def tok_block(off, sz):
        return tokp.ap()[D * off : D * (off + sz)].rearrange(
            "(ki r) -> ki r", ki=P
        )

    def out_block(off, sz):
        return outp.ap()[D * off : D * (off + sz)].rearrange(
            "(mi r) -> mi r", mi=P
        )

    with TileContext(nc) as tc:
        with (
            tc.tile_pool(name="w", bufs=1) as w_pool,
            tc.tile_pool(name="tok", bufs=2) as tok_pool,
            tc.tile_pool(name="out", bufs=2) as out_pool,
            tc.tile_pool(name="bias", bufs=1) as b_pool,
            tc.tile_pool(name="ps", bufs=8, space="PSUM") as ps_pool,
        ):
            bias_tile = b_pool.tile([P, nsets, MT], mybir.dt.float32)
            tok_tiles = {}

            def load_toks(n, n_off, n_sz, chunks=None, eng=None):
                # flat [P, KT*sz] tile: k-slice c is columns [c*sz, (c+1)*sz)
                t_full = tok_pool.tile([P, KT * 512], dt_in, tag="tok")
                t_tile = t_full[:, : KT * n_sz]
                blk = tok_block(n_off, n_sz)
                eng = eng or nc.sync
                if chunks:
                    # k-slice chunks so the first matmuls only wait for the
                    # slices they read
                    k = 0
                    for w in chunks:
                        eng.dma_start(
                            t_tile[:, k * n_sz : (k + w) * n_sz],
                            blk[:, k * n_sz : (k + w) * n_sz],
                        )
                        k += w
                else:
                    eng.dma_start(t_tile, blk)
                tok_tiles[n] = (t_tile, n_sz)

            w_tiles = [[None] * MT for _ in range(nsets)]

            def load_w_batch(s, lo, hi):
                w_t = w_pool.tile([P, hi - lo, KT, P], dt_in, tag=f"w{s}_{lo}")
                nc.scalar.dma_start(w_t[:], w_r[s][:, lo:hi])
                for m in range(lo, hi):
                    w_tiles[s][m] = w_t[:, m - lo]

            # issue order ~= consumption order. Act queue: the weight
            # stream, with w0 split at k=8 so the first matmuls only gate
            # on a 0.26MB download. SP queue: first token tile (k-chunked,
            # small chunk first so matmul 0 starts ASAP), bias, then (from
            # the compute loop) per-tile token prefetches and output DMAs.
            w0_t = w_pool.tile([P, 1, KT, P], dt_in, tag="w0_0")
            nc.scalar.dma_start(w0_t[:, :, : KT // 2], w_r[0][:, 0:1, : KT // 2])
            nc.scalar.dma_start(w0_t[:, :, KT // 2 :], w_r[0][:, 0:1, KT // 2 :])
            w_tiles[0][0] = w0_t[:, 0]
            load_toks(0, 0, tiles[0][0], chunks=(4, 12))
            nc.sync.dma_start(bias_tile[:], b_r)
            for lo, hi in _W_SET0[1:]:
                load_w_batch(0, lo, hi)
            if nsets > 1:
                for lo, hi in _W_SET1:
                    load_w_batch(1, lo, hi)

            n_offs = []
            off = 0
            for sz, _s in tiles:
                n_offs.append(off)
                off += sz

            for n, (n_sz, s) in enumerate(tiles):
                if n >= 2 and n + 1 < len(tiles):
                    load_toks(n + 1, n_offs[n + 1], tiles[n + 1][0])
                t_tile, t_sz = tok_tiles.pop(n)
                assert t_sz == n_sz
                n_off = n_offs[n]
                o_full = out_pool.tile([P, MT * 512], dt_out, tag="out")
                o_tile = o_full[:, : MT * n_sz]
                for m in range(MT):
                    # issue the n=1 token prefetch from the middle of n=0's
                    # SP stream: early enough to land before n=1 starts,
                    # late enough not to steal HBM from the weight stream
                    if n == 0 and m == 8 and len(tiles) > 1:
                        load_toks(1, n_offs[1], tiles[1][0])
                    if n == 1 and m == 8 and len(tiles) > 2:
                        load_toks(2, n_offs[2], tiles[2][0])
                    ps_full = ps_pool.tile([P, 512], mybir.dt.float32, tag="ps")
                    ps = ps_full[:, :n_sz]
                    for k in range(KT):
                        nc.tensor.matmul(
                            ps,
                            w_tiles[s][m][:, k, :],
                            t_tile[:, k * n_sz : (k + 1) * n_sz],
                            start=(k == 0),
                            stop=(k == KT - 1),
                        )
                    nc.vector.tensor_scalar_add(
                        o_tile[:, m * n_sz : (m + 1) * n_sz],
                        ps,
                        bias_tile[:, s, m : m + 1],
                    )
                # ONE output DMA per tile (not per m): DMA count is the
                # scarce resource. The last tile's outputs go in 4 chunks
                # so the final drain pipelines behind the last matmuls.
                o_blk = out_block(n_off, n_sz)
                if n == len(tiles) - 1:
                    for m4 in range(0, MT, 4):
                        nc.sync.dma_start(
                            o_blk[:, m4 * n_sz : (m4 + 4) * n_sz],
                            o_tile[:, m4 * n_sz : (m4 + 4) * n_sz],
                        )
                else:
                    nc.sync.dma_start(o_blk, o_tile)
    nc.compile()
    return nc


def _get_program(Ca, Cb, dt_name):
    key = (Ca, Cb, dt_name)
    if key not in _cache:
        _cache[key] = _build(Ca, Cb, dt_name)
    return _cache[key]


# ------------------------------------------------------------- host: pack ---


def _solve_pack(counts):
    """2-weight-set slot packing: each core gets slot_a (Ca cols, weight
    set 0) and slot_b (Cb cols, weight set 1); every slot holds a chunk of
    ONE expert's tokens. x experts split across two a-slots, 8-2x experts
    use one core's (a,b), x experts split across two b-slots. Minimizes
    N = Ca + Cb (the per-core padded column count)."""
    n = len(counts)
    order = np.argsort(-np.asarray(counts), kind="stable")
    c = [int(counts[i]) for i in order]
    best = None
    for x in range(0, n // 2 + 1):
        if x == 0:
            a, b, N = c[0], 0, c[0]
        else:
            a = (c[0] + 1) // 2
            b = (c[n - x] + 1) // 2
            mids = c[x : n - x]
            N = max(a + b, mids[0] if mids else 0)
            a = max(a, N - b)
            b = N - a
        if best is None or N < best[0]:
            best = (N, x, a, b)
    N, x, a, b = best
    plan = [[] for _ in range(n)]  # per core: list of (slot, expert, lo, hi)
    for j in range(x):  # biggest experts -> a-slots of cores 2j, 2j+1
        e = int(order[j])
        ce = c[j]
        cut = min(a, ce)
        plan[2 * j].append(("a", e, 0, cut))
        if ce > cut:
            plan[2 * j + 1].append(("a", e, cut, ce))
    for k, j in enumerate(range(x, n - x)):  # mids -> (a,b) of one core
        e = int(order[j])
        ce = c[j]
        core = 2 * x + k
        cut = min(a, ce)
        plan[core].append(("a", e, 0, cut))
        if ce > cut:
            plan[core].append(("b", e, cut, ce))
    for j in range(x):  # smallest experts -> b-slots of cores 2j, 2j+1
        e = int(order[n - x + j])
        ce = c[n - x + j]
        cut = min(b, ce)
        plan[2 * j].append(("b", e, 0, cut))
        if ce > cut:
            plan[2 * j + 1].append(("b", e, cut, ce))
    # sanity: coverage and capacity
    cov = [0] * n
    for core, items in enumerate(plan):
        used = {"a": 0, "b": 0}
        owners = {"a": set(), "b": set()}
        for slot, e, lo, hi in items:
            used[slot] += hi - lo
            owners[slot].add(e)
            cov[e] += hi - lo
        if used["a"] > a or used["b"] > b:
            return None
        if len(owners["a"]) > 1 or len(owners["b"]) > 1:
            return None
    if cov != [int(v) for v in counts]:
        return None
    return N, x, a, b, plan


# ------------------------------------------------------------------- host ---


def kernel(x, y, W_experts, b_experts, W_gate, b_gate):
    x = np.asarray(x, dtype=np.float32)
    y = np.asarray(y, dtype=np.float32)
    W_experts = np.asarray(W_experts, dtype=np.float32)
    b_experts = np.asarray(b_experts, dtype=np.float32)
    W_gate = np.asarray(W_gate, dtype=np.float32)
    b_gate = np.asarray(b_gate, dtype=np.float32)

    inp = np.concatenate([x, y], axis=1)  # [B, D]

    # ---- routing (host) ----
    logits = inp.astype(np.float64) @ W_gate.T.astype(np.float64) + b_gate
    order = np.argsort(-logits, axis=1, kind="stable")
    top2 = order[:, :TOPK]  # [B, 2]
    v = np.take_along_axis(logits, top2, axis=1)
    v = v - v.max(axis=1, keepdims=True)
    ev = np.exp(v)
    g = (ev / ev.sum(axis=1, keepdims=True)).astype(np.float32)  # [B, 2]

    counts = np.bincount(top2.ravel(), minlength=E)

    idx_list = []
    wgt_list = []
    for e in range(E):
        m0 = top2[:, 0] == e
        m1 = top2[:, 1] == e
        idx_e = np.concatenate([np.nonzero(m0)[0], np.nonzero(m1)[0]])
        w_e = np.concatenate([g[m0, 0], g[m1, 1]])
        idx_list.append(idx_e)
        wgt_list.append(w_e)

    dt_name = os.environ.get("MOE_DT", "bfloat16")
    if dt_name == "bfloat16":
        import ml_dtypes

        np_in_dt = np.dtype(ml_dtypes.bfloat16)
    else:
        np_in_dt = np.dtype(np.float32)

    pack = None
    if os.environ.get("MOE_PACK", "1") == "1" and E == N_CORES:
        pack = _solve_pack(counts)
    if pack is not None and pack[3] > 0:
        N, _x, Ca, Cb, plan = pack
        Ca = max(Ca, 512)  # tiling floor
    else:
        Ca, Cb = max(512, int(counts.max())), 0
        plan = [[("a", e, 0, int(counts[e]))] for e in range(E)]

    inpT = np.ascontiguousarray(inp.T.astype(np_in_dt))  # [D, B]
    MT = KT = D // P
    wpacked = [None] * E

    def _wpack(e):
        if wpacked[e] is None:
            # wp[ki, m, ko, o] = W_e[m*128 + o, ko*128 + ki] (partition-major)
            wpacked[e] = np.ascontiguousarray(
                W_experts[e]
                .reshape(MT, P, KT, P)
                .transpose(3, 0, 2, 1)
                .astype(np_in_dt)
            )
        return wpacked[e]

    # device-side tile structure (must mirror _split_slot in _build)
    tile_sizes = [sz for sz in _split_slot(Ca, lead_small=True)]
    tile_offs = list(np.cumsum([0] + tile_sizes))[:-1]
    if Cb:
        b_sizes = _split_slot(Cb, tail_small=True)
        tile_offs += [Ca + o for o in np.cumsum([0] + b_sizes)[:-1]]
        tile_sizes += b_sizes

    def _tok_pack(toksT):
        # [D, C] -> tile-blocked [P, KT, sz] per tile, flattened
        blocks = []
        for off, sz in zip(tile_offs, tile_sizes):
            blk = toksT[:, off : off + sz].reshape(KT, P, sz).transpose(1, 0, 2)
            blocks.append(np.ascontiguousarray(blk).reshape(-1))
        return np.concatenate(blocks)

    nsets = 2 if Cb else 1
    slot_off = {"a": 0, "b": Ca}
    slot_idx = {"a": 0, "b": 1}
    in_maps = []
    for core in range(E):
        toksT = np.zeros((D, Ca + Cb), dtype=np_in_dt)
        wp = np.zeros((nsets, P, MT, KT, P), dtype=np_in_dt)
        bias = np.zeros((nsets, D), dtype=np.float32)
        for slot, e, lo, hi in plan[core]:
            off = slot_off[slot]
            toksT[:, off : off + (hi - lo)] = inpT[:, idx_list[e][lo:hi]]
            wp[slot_idx[slot]] = _wpack(e)
            bias[slot_idx[slot]] = b_experts[e]
        # biasp[mi, s, mo] = bias[s, mo*128 + mi] (partition-major)
        biasp = np.ascontiguousarray(
            bias.reshape(nsets, MT, P).transpose(2, 0, 1)
        )
        in_maps.append({"wp": wp, "tokp": _tok_pack(toksT), "bias": biasp})

    # ---- device ----
    if os.environ.get("BASS_TRACE"):
        _ntff_shim()
    from concourse.bass_utils import run_bass_kernel_spmd

    nc = _get_program(Ca, Cb, dt_name)
    res = None
    for attempt in range(3):
        try:
            res = run_bass_kernel_spmd(nc, in_maps, core_ids=list(range(N_CORES)))
            break
        except Exception:
            # the axon-tunneled device occasionally reports a transient
            # NRT_EXEC_UNIT_UNRECOVERABLE; it recovers after a short wait
            if attempt == 2:
                raise
            import time

            time.sleep(20 * (attempt + 1))
            try:
                import jax

                jax.clear_caches()
            except Exception:
                pass
    globals()["_last_res"] = res
    if res.exec_time_ns is not None:
        print(f"HW exec time: {res.exec_time_ns} ns")

    # ---- combine (host) ----
    fused = np.zeros((B, D), dtype=np.float32)
    for core in range(E):
        outp = res.results[core]["outp"]
        # un-block: tile n is [P, MT, sz] at offset D*tile_offs[n]
        outT = np.empty((D, Ca + Cb), dtype=np.float32)
        for off, sz in zip(tile_offs, tile_sizes):
            blk = outp[D * off : D * (off + sz)].reshape(P, MT, sz)
            outT[:, off : off + sz] = (
                blk.transpose(1, 0, 2).reshape(D, sz).astype(np.float32)
            )
        for slot, e, lo, hi in plan[core]:
            off = slot_off[slot]
            rows = outT[:, off : off + (hi - lo)].T
            fused[idx_list[e][lo:hi]] += rows * wgt_list[e][lo:hi, None]
    return fused
